# revision 16
# baseline (speedup 1.0000x reference)
"""Gaussian-kernel layer (exp(-||x - w_m||^2) + b_m) as a Bass/Tile TRN2 kernel.

Math (per row n of x, per center m):
    out[n, m] = exp(-(x2[n] + w2[m] - 2*x.w)) + b[m]
              = exp(2*(xw[n,m] - w2[m]/2) - x2[n]) + b[m]

Mapping onto the NeuronCore:
  - data-parallel over batch: 16 batches -> 2 per core on 8 cores
  - per core: 4608 rows x 128 ch -> 36 row-tiles of 128
  - PE: per-tile transpose of x (fp32), then bf16 matmul
        P = x_t.T @ w  accumulated on top of a K=1 matmul that
        pre-loads -w2/2 broadcast along rows
  - ACT: e = Exp(2*P + bias) with per-partition bias = -x2[n]
  - DVE: x2 via fused square+reduce (tensor_tensor_reduce), final + b
  - output written as fp32, exact b + exp contribution
"""

from contextlib import ExitStack

import numpy as np

import concourse.bacc as bacc
import concourse.bass as bass
import concourse.mybir as mybir
import concourse.tile as tile
from concourse.bass_utils import run_bass_kernel_spmd
from concourse.masks import make_identity

B, H, W_, C, M = 16, 48, 48, 128, 512
N_CORES = 8
B_PER = B // N_CORES          # 2 batches per core
ROWS = B_PER * H * W_         # 4608 rows per core
P = 128                       # partition / row-tile size
N_TILES = ROWS // P           # 36

F32 = mybir.dt.float32
BF16 = mybir.dt.bfloat16

_NC_CACHE = {}


def _build_nc():
    nc = bacc.Bacc(
        "TRN2",
        target_bir_lowering=False,
        debug=False,
        num_devices=N_CORES,
    )
    x_d = nc.declare_dram_parameter("x", [ROWS, C], F32, isOutput=False)
    w_d = nc.declare_dram_parameter("w", [C, M], F32, isOutput=False)
    b_d = nc.declare_dram_parameter("b", [1, M], F32, isOutput=False)
    o_d = nc.declare_dram_parameter("out", [ROWS, M], F32, isOutput=True)

    AF = mybir.ActivationFunctionType

    with tile.TileContext(nc) as tc, ExitStack() as ctx:
        consts = ctx.enter_context(tc.tile_pool(name="consts", bufs=1))
        pool = ctx.enter_context(tc.tile_pool(name="work", bufs=4))
        epool = ctx.enter_context(tc.tile_pool(name="exp", bufs=3))
        opool = ctx.enter_context(tc.tile_pool(name="outp", bufs=4))
        ps_t = ctx.enter_context(
            tc.tile_pool(name="ps_t", bufs=2, space=bass.MemorySpace.PSUM)
        )
        ps_mm = ctx.enter_context(
            tc.tile_pool(name="ps_mm", bufs=4, space=bass.MemorySpace.PSUM)
        )
        ps_one = ctx.enter_context(
            tc.tile_pool(name="ps_one", bufs=1, space=bass.MemorySpace.PSUM)
        )

        # ---- one-time constants ----
        w_sb = consts.tile([C, M], F32)
        nc.sync.dma_start(w_sb[:], w_d[:])
        b_sb = consts.tile([1, M], F32)
        nc.sync.dma_start(b_sb[:], b_d[:])

        w_bf = consts.tile([C, M], BF16)
        nc.vector.tensor_copy(w_bf[:], w_sb[:])

        ident = consts.tile([P, P], F32)
        make_identity(nc, ident[:])

        ones_c = consts.tile([C, 1], F32)
        nc.gpsimd.memset(ones_c[:], 1.0)
        ones_r_bf = consts.tile([1, P], BF16)
        nc.gpsimd.memset(ones_r_bf[:], 1.0)
        ones_r_f = consts.tile([1, P], F32)
        nc.gpsimd.memset(ones_r_f[:], 1.0)

        # w2[m] = sum_c w[c,m]^2 via ones.T @ (w*w); v = -w2/2 (bf16)
        wsq = consts.tile([C, M], F32)
        nc.vector.tensor_mul(wsq[:], w_sb[:], w_sb[:])
        p_w2 = ps_one.tile([P, M], F32, tag="ps_pre")
        nc.tensor.matmul(p_w2[:1, :], ones_c[:], wsq[:], start=True, stop=True)
        v_bf = consts.tile([1, M], BF16)
        nc.scalar.activation(v_bf[:], p_w2[:1, :], AF.Copy, scale=-0.5)

        # bb[p, m] = b[m] broadcast along partitions (exact fp32: 1.0 * b)
        p_bb = ps_one.tile([P, M], F32, tag="ps_pre")
        nc.tensor.matmul(p_bb[:], ones_r_f[:], b_sb[:], start=True, stop=True)
        bb = consts.tile([P, M], F32)
        nc.vector.tensor_copy(bb[:], p_bb[:])

        # bb4: b broadcast over partitions, repeated 4x along free
        bb4 = consts.tile([P, 4, M], F32)
        for j in range(4):
            nc.vector.tensor_copy(bb4[:, j, :], bb[:])

        # PE warm-up: ~4us of dense back-to-back dummy matmuls so the
        # HAM clock-gate opens (1.2 -> 2.4 GHz) before the main loop.
        # Overlaps the preamble DMAs; results are never read.
        warm_w = consts.tile([C, M], BF16)
        nc.gpsimd.memset(warm_w[:], 0.0)
        p_warm = ps_one.tile([P, M], F32, tag="p_warm")
        for _ in range(10):
            nc.tensor.matmul(p_warm[:], warm_w[:, :P], warm_w[:], start=True,
                             stop=True)

        # ---- main loop: 9 groups of 4 row-tiles ----
        # DMA queues by issuing engine: gpsimd=SWDGE q0, sync=HWDGE q1,
        # scalar=HWDGE q10. Inputs on gpsimd; outputs split to balance
        # bytes per queue (~4MB each).
        G = 4
        N_G = N_TILES // G  # 9
        x_v = x_d.rearrange("(g j p) c -> g p j c", j=G, p=P)
        o_v = o_d.rearrange("(g j p) m -> g p j m", j=G, p=P)
        out_engs = [nc.sync, nc.sync, nc.scalar, nc.sync,
                    nc.sync, nc.scalar, nc.sync, nc.sync, nc.scalar]
        for g in range(N_G):
            x_nat4 = pool.tile([P, G, C], F32, tag="x_nat4")
            nc.gpsimd.dma_start(x_nat4[:], x_v[g])

            x_t2s = []
            for jj in range(G // 2):
                p_t2 = ps_t.tile([C, 2 * P], F32, tag="p_t2")
                nc.tensor.transpose(p_t2[:, :P], x_nat4[:, 2 * jj, :], ident[:])
                nc.tensor.transpose(
                    p_t2[:, P:], x_nat4[:, 2 * jj + 1, :], ident[:]
                )
                x_t2 = pool.tile([C, 2 * P], BF16, tag="x_t2")
                nc.scalar.activation(x_t2[:], p_t2[:], AF.Copy)
                x_t2s.append(x_t2)

            e4 = epool.tile([P, G, M], F32, tag="e4")
            for j in range(G):
                sq = pool.tile([P, C], F32, tag="sq")
                negx2 = pool.tile([P, 1], F32, tag="negx2")
                nc.vector.scalar_tensor_tensor(
                    out=sq[:],
                    in0=x_nat4[:, j, :],
                    scalar=-1.0,
                    in1=x_nat4[:, j, :],
                    op0=mybir.AluOpType.mult,
                    op1=mybir.AluOpType.mult,
                    accum_out=negx2[:],
                )

                p_mm = ps_mm.tile([P, M], F32, tag="p_mm")
                nc.tensor.matmul(
                    p_mm[:], ones_r_bf[:], v_bf[:], start=True, stop=False
                )
                xt = x_t2s[j // 2]
                nc.tensor.matmul(
                    p_mm[:],
                    xt[:, (j % 2) * P : (j % 2 + 1) * P],
                    w_bf[:],
                    start=False,
                    stop=True,
                )

                nc.scalar.activation(
                    e4[:, j, :], p_mm[:], AF.Exp, bias=negx2[:], scale=2.0
                )

            o_t4 = opool.tile([P, G, M], F32, tag="o_t4")
            nc.vector.tensor_add(o_t4[:], e4[:], bb4[:])
            if g == N_G - 1:
                # split the last store across both HWDGE queues to
                # halve the kernel-tail DMA
                nc.sync.dma_start(o_v[g][:, : G // 2, :], o_t4[:, : G // 2, :])
                nc.scalar.dma_start(o_v[g][:, G // 2 :, :], o_t4[:, G // 2 :, :])
            else:
                out_engs[g].dma_start(o_v[g], o_t4[:])

    nc.compile()
    return nc


def _get_nc():
    if "nc" not in _NC_CACHE:
        _NC_CACHE["nc"] = _build_nc()
    return _NC_CACHE["nc"]


def _run(x, w, b, trace=False, tmpdir=None):
    nc = _get_nc()
    xs = np.ascontiguousarray(np.asarray(x, dtype=np.float32)).reshape(
        N_CORES, ROWS, C
    )
    wf = np.ascontiguousarray(np.asarray(w, dtype=np.float32))
    bf = np.ascontiguousarray(np.asarray(b, dtype=np.float32)).reshape(1, M)
    in_maps = [{"x": xs[i], "w": wf, "b": bf} for i in range(N_CORES)]
    res = run_bass_kernel_spmd(
        nc, in_maps, list(range(N_CORES)), trace=trace, tmpdir=tmpdir
    )
    out = np.stack([res.results[i]["out"] for i in range(N_CORES)], axis=0)
    return out.reshape(B, H * W_, M), res


def kernel(x, w, b):
    out, _ = _run(x, w, b, trace=False)
    return out
